# revision 40
# baseline (speedup 1.0000x reference)
"""Multi-head attention block (B=4, N=2048, D=768, H=12) on 8 TRN2 NeuronCores.

Sharding: core i -> batch b = i//2, query-row half qh = i%2 (1024 rows).
Each core computes all 12 heads for its (batch, q-half): qkv projection,
flash-style attention in S^T layout (keys on partitions, queries on free
axis), and the full output projection for its rows. No collectives: every
core produces complete output rows; host just concatenates.

Matmul inputs are bf16 (1 cyc/row on PE vs 4 for fp32); all accumulation is
fp32 in PSUM. Softmax skips the max-subtraction (scores are ~N(0,1); exp is
exact in fp32 for this range). Row-sums come from a ones-column appended to
V so the O-matmul accumulates them for free.

The emission order interleaves qkv column-tile work into the ACT-bound
attention kt-loops so the PE stream stays dense (keeps the HAM clock at
2.4 GHz) while the Scalar engine streams the exps.
"""

import sys

sys.path.insert(0, "/opt/trn_rl_repo")

import numpy as np
import ml_dtypes

import concourse.bass as bass
import concourse.tile as tile
from concourse import bacc, mybir
from concourse.bass_utils import run_bass_kernel_spmd
from concourse.masks import make_identity

B, N, D, H = 4, 2048, 768, 12
HD = D // H  # 64
SCALE = HD**-0.5
NCORES = 8
QR = N // 2  # q rows per core
KT = D // 128  # 6 contraction tiles
NT = N // 128  # 16 key-row tiles
BF = mybir.dt.bfloat16
F32 = mybir.dt.float32

TRACE = False
LAST_EXEC_NS = None
_CACHED_NC = None


def _body(tc, xt, xtq, wqkv, wproj, biasb, y, qattn):
    nc = tc.nc
    with (
        tc.tile_pool(name="const", bufs=1) as cp,
        tc.tile_pool(name="persist", bufs=1) as pp,
        # all PSUM lives in one scope: mm 2 banks + st 4 banks + o 2 banks
        tc.tile_pool(name="mmps", bufs=2, space="PSUM") as mmp,
        tc.tile_pool(name="stps", bufs=2, space="PSUM") as stp,
        tc.tile_pool(name="ops", bufs=1, space="PSUM") as op_,
        tc.tile_pool(name="work", bufs=3) as sb2,
    ):
        ones_bf = cp.tile([1, 128], BF)
        nc.vector.memset(ones_bf[:], 1.0)
        ident = cp.tile([128, 128], F32)
        make_identity(nc, ident[:])

        def emit_bias_bc():
            for n0, nw in [(0, 512), (512, 256)]:
                ps = mmp.tile([128, 512], F32, tag="mm", name="biasps")
                nc.tensor.matmul(
                    ps[:, :nw],
                    lhsT=ones_bf[:1, :128],
                    rhs=bias_sb[:1, n0 : n0 + nw],
                    start=True,
                    stop=True,
                )
                nc.vector.tensor_copy(bias_bc[:, n0 : n0 + nw], ps[:, :nw])
        bias_sb = cp.tile([1, D], BF)
        nc.sync.dma_start(bias_sb[:], biasb[:])
        bias_bc = cp.tile([128, D], F32)

        xt_sb = [pp.tile([128, N], BF, tag=f"xt{k}", name=f"xt{k}") for k in range(KT)]
        xtq_sb = [pp.tile([128, QR], BF, tag=f"xtq{k}", name=f"xtq{k}") for k in range(KT)]
        w_sb = [pp.tile([128, 3 * D], BF, tag=f"w{k}", name=f"w{k}") for k in range(KT)]
        wp_sb = [pp.tile([128, D], BF, tag=f"wp{k}", name=f"wp{k}") for k in range(KT)]
        qT_sb = [pp.tile([128, QR], BF, tag=f"qT{k}", name=f"qT{k}") for k in range(KT)]
        kT_sb = [pp.tile([128, N], BF, tag=f"kT{k}", name=f"kT{k}") for k in range(KT)]
        v_sb = [pp.tile([128, H * (HD + 1)], BF, tag=f"v{r}", name=f"v{r}") for r in range(NT)]
        oT_sb = [pp.tile([128, QR], BF, tag=f"oT{k}", name=f"oT{k}") for k in range(KT)]
        qattn_sb = pp.tile([16, H * 128], F32, tag="qattn_sb")

        # all input loads on the sync HWDGE queue, in strict need-first order:
        # the first attention head gates only on xtq, w cols [0:128] (q col 0),
        # w cols [768:896] (k col 0), and xt -- 5.0MB; the rest trails.
        for k in range(KT):
            ks = slice(k * 128, (k + 1) * 128)
            nc.sync.dma_start(xtq_sb[k][:], xtq[ks, :])
            nc.sync.dma_start(w_sb[k][:, 0:128], wqkv[ks, 0:128])
            nc.sync.dma_start(w_sb[k][:, D : D + 128], wqkv[ks, D : D + 128])
        for k in range(KT):
            ks = slice(k * 128, (k + 1) * 128)
            nc.sync.dma_start(xt_sb[k][:], xt[ks, :])
        for k in range(KT):
            ks = slice(k * 128, (k + 1) * 128)
            nc.sync.dma_start(w_sb[k][:, 128:D], wqkv[ks, 128:D])
            nc.sync.dma_start(w_sb[k][:, D + 128 : 2 * D], wqkv[ks, D + 128 : 2 * D])
        for k in range(KT):
            ks = slice(k * 128, (k + 1) * 128)
            nc.sync.dma_start(w_sb[k][:, 2 * D :], wqkv[ks, 2 * D :])
        for k in range(KT):
            ks = slice(k * 128, (k + 1) * 128)
            nc.sync.dma_start(wp_sb[k][:], wproj[ks, :])

        # ---- emit helpers ----

        def _drain(filler):
            while filler:
                try:
                    next(filler)()
                except StopIteration:
                    break

        def v_groups(r):
            # v in [row, feature] layout, ones column appended per head
            hv = v_sb[r][:].rearrange("p (h c) -> p h c", c=HD + 1)
            for gi, (c0, cw) in enumerate([(2 * D, 512), (2 * D + 512, 256)]):
                def emit(gi=gi, c0=c0, cw=cw):
                    if gi == 0:
                        nc.gpsimd.memset(hv[:, :, HD : HD + 1], 1.0)
                    ps = mmp.tile([128, 512], F32, tag="mm", name="vps")
                    for k in range(KT):
                        nc.tensor.matmul(
                            ps[:, :cw],
                            lhsT=xt_sb[k][:, r * 128 : (r + 1) * 128],
                            rhs=w_sb[k][:, c0 : c0 + cw],
                            start=(k == 0),
                            stop=(k == KT - 1),
                        )
                    h0 = (c0 - 2 * D) // HD
                    nc.vector.tensor_copy(
                        hv[:, h0 : h0 + cw // HD, 0:HD],
                        ps[:, :cw].rearrange("p (h c) -> p h c", c=HD),
                    )
                yield emit

        def qk_groups(c):
            # generator of closures: one psum-group (6 matmuls + copy) each
            is_q = c < 6
            rhs_tiles = xtq_sb if is_q else xt_sb
            nrows = QR if is_q else N
            dst = qT_sb[c] if is_q else kT_sb[c - 6]
            for n0 in range(0, nrows, 512):
                def emit(n0=n0):
                    ps = mmp.tile([128, 512], F32, tag="mm", name="qkps")
                    for k in range(KT):
                        nc.tensor.matmul(
                            ps[:],
                            lhsT=w_sb[k][:, c * 128 : (c + 1) * 128],
                            rhs=rhs_tiles[k][:, n0 : n0 + 512],
                            start=(k == 0),
                            stop=(k == KT - 1),
                        )
                    nc.vector.tensor_copy(dst[:, n0 : n0 + 512], ps[:])
                yield emit

        def emit_head(h, filler, every=3, per=1):
            # flash attention for one head; calls filler closures between kt
            # steps to keep the PE stream dense while ACT runs the exps.
            # Returns tail closures (PE-touching normalize/q_attn work) that
            # the caller should feed into the NEXT head's filler stream so
            # the PE queue never waits on the DVE reciprocal chain.
            t6 = h // 2
            po = (h % 2) * 64
            qT_h = qT_sb[t6][po : po + 64, :]  # [64, QR]
            o_ps = [
                op_.tile([HD + 1, 512], F32, tag=f"o{qc}", name=f"o{qc}")
                for qc in range(2)
            ]
            stage = sb2.tile([128, NT], F32, tag="stage")
            for kt in range(NT):
                st = stp.tile([128, QR], F32, tag="st")
                lhsT = kT_sb[t6][po : po + 64, kt * 128 : (kt + 1) * 128]
                for qc in range(2):
                    nc.tensor.matmul(
                        st[:, qc * 512 : (qc + 1) * 512],
                        lhsT=lhsT,
                        rhs=qT_h[:, qc * 512 : (qc + 1) * 512],
                        start=True,
                        stop=True,
                    )
                pT = sb2.tile([128, QR], BF, tag="pT", bufs=4)
                nc.scalar.activation(
                    pT[:], st[:], mybir.ActivationFunctionType.Exp, scale=SCALE
                )
                # filler fires BEFORE the O-matmuls: a filler writing v_sb[j]
                # emitted at kt=j still precedes O(j) in program order
                if kt % every == 0 and filler:
                    for _ in range(per):
                        try:
                            next(filler)()
                        except StopIteration:
                            filler = None
                            break
                vh = v_sb[kt][:, h * (HD + 1) : (h + 1) * (HD + 1)]  # [128, 65]
                for qc in range(2):
                    nc.tensor.matmul(
                        o_ps[qc][:],
                        lhsT=vh,
                        rhs=pT[:, qc * 512 : (qc + 1) * 512],
                        start=(kt == 0),
                        stop=(kt == NT - 1),
                    )
                nc.gpsimd.tensor_copy(stage[:, kt : kt + 1], pT[:, 0:1])
            # evacuate o_ps fast (frees accumulator banks for the next head)
            # and run the DVE-only reciprocal chain now; defer PE-touching
            # normalize/q_attn ops as closures.
            o_raw = sb2.tile([HD + 1, QR], F32, tag="o_raw", bufs=2)
            for qc in range(2):
                nc.vector.tensor_copy(o_raw[:, qc * 512 : (qc + 1) * 512], o_ps[qc][:])
            rowsum = sb2.tile([1, QR], F32, tag="rowsum")
            nc.vector.tensor_copy(rowsum[:], o_raw[HD : HD + 1, :])
            recip = sb2.tile([1, QR], F32, tag="recip")
            nc.vector.reciprocal_approx_fast(recip[:], rowsum[:])
            recip_bf = sb2.tile([1, QR], BF, tag="recip_bf")
            nc.vector.tensor_copy(recip_bf[:], recip[:])

            scale_sb = sb2.tile([128, 1], F32, tag="scale_sb")

            def norm_qc(qc):
                def emit():
                    bc = mmp.tile([128, 512], F32, tag="mm", name="bc")
                    nc.tensor.matmul(
                        bc[:],
                        lhsT=ones_bf[:1, :128],
                        rhs=recip_bf[:1, qc * 512 : (qc + 1) * 512],
                        start=True,
                        stop=True,
                    )
                    nc.vector.tensor_mul(
                        oT_sb[t6][po : po + 64, qc * 512 : (qc + 1) * 512],
                        o_raw[0:HD, qc * 512 : (qc + 1) * 512],
                        bc[0:HD, :],
                    )
                    if qc == 0:
                        # column 0 = 1/rowsum(q=0) on all 128 partitions:
                        # doubles as the q_attn scale factor
                        nc.vector.tensor_copy(scale_sb[:], bc[:, 0:1])
                return emit

            def qattn_scale():
                # q_attn: query row 0's softmax row (valid on qh==0 cores)
                nc.vector.tensor_scalar_mul(stage[:], stage[:], scale_sb[:])

            def qattn_store():
                tp = mmp.tile([128, 512], F32, tag="mm", name="tp")
                nc.tensor.transpose(tp[0:NT, 0:128], stage[:], ident[:])
                nc.vector.tensor_copy(
                    qattn_sb[:, h * 128 : (h + 1) * 128], tp[0:NT, 0:128]
                )

            return filler, [norm_qc(0), norm_qc(1), qattn_scale, qattn_store]

        # ---- emission: first q/k columns + 3 v tiles, then heads; filler
        # streams the remaining v tiles, prior head's tail, next q/k cols ----
        qk0 = list(qk_groups(0))
        qk6 = list(qk_groups(6))
        for g in [*qk0, qk6[0]]:
            g()
        pending = []
        for p in range(6):
            for h in (2 * p, 2 * p + 1):
                if h == 0:
                    fl = iter(
                        [
                            *v_groups(0),
                            *v_groups(1),
                            *qk6[1:],
                            *(g for r in range(2, NT) for g in v_groups(r)),
                            emit_bias_bc,
                        ]
                    )
                    fl, tail = emit_head(0, fl, every=1, per=3)
                    _drain(fl)
                    pending = [*tail, *qk_groups(1), *qk_groups(7)]
                else:
                    if h % 2 == 0 and p < 5:
                        pending += [*qk_groups(p + 1), *qk_groups(7 + p)]
                    fl = iter(pending)
                    fl, tail = emit_head(h, fl, every=2, per=1)
                    leftover = list(fl) if fl else []
                    pending = leftover + list(tail)
        for g in pending:
            g()

        nc.sync.dma_start(
            qattn.rearrange("h (t c) -> t h c", c=128),
            qattn_sb[:].rearrange("p (h c) -> p h c", c=128),
        )

        # ---- output projection (+bias via ones row) ----
        for r in range(QR // 128):
            ytile = sb2.tile([128, D], BF, tag="y", bufs=2)
            for gi, (n0, nw) in enumerate([(0, 512), (512, 256)]):
                pool, tg = [(mmp, "mm"), (op_, "o0"), (op_, "o1")][(2 * r + gi) % 3]
                ps = pool.tile([128, 512], F32, tag=tg, name="pjps")
                for k in range(KT):
                    nc.tensor.matmul(
                        ps[:, :nw],
                        lhsT=oT_sb[k][:, r * 128 : (r + 1) * 128],
                        rhs=wp_sb[k][:, n0 : n0 + nw],
                        start=(k == 0),
                        stop=(k == KT - 1),
                    )
                nc.vector.tensor_add(
                    ytile[:, n0 : n0 + nw], ps[:, :nw], bias_bc[:, n0 : n0 + nw]
                )
            nc.sync.dma_start(y[r * 128 : (r + 1) * 128, :], ytile[:])


def _build():
    nc = bacc.Bacc("TRN2", target_bir_lowering=False, debug=False, num_devices=NCORES)
    xt = nc.declare_dram_parameter("xt", [D, N], BF, isOutput=False)
    xtq = nc.declare_dram_parameter("xtq", [D, QR], BF, isOutput=False)
    wqkv = nc.declare_dram_parameter("wqkv", [D, 3 * D], BF, isOutput=False)
    wproj = nc.declare_dram_parameter("wproj", [D, D], BF, isOutput=False)
    biasb = nc.declare_dram_parameter("biasb", [1, D], BF, isOutput=False)
    y = nc.declare_dram_parameter("y", [QR, D], BF, isOutput=True)
    qattn = nc.declare_dram_parameter("qattn", [H, N], F32, isOutput=True)
    with tile.TileContext(nc) as tc:
        _body(tc, xt.ap(), xtq.ap(), wqkv.ap(), wproj.ap(), biasb.ap(), y.ap(), qattn.ap())
    nc.finalize()
    return nc


def kernel(x, w_qkv, w_proj, b_proj):
    global _CACHED_NC, LAST_EXEC_NS
    if _CACHED_NC is None:
        _CACHED_NC = _build()
    nc = _CACHED_NC

    bf16 = ml_dtypes.bfloat16
    x = np.asarray(x, np.float32)
    wqkv_bf = np.ascontiguousarray(np.asarray(w_qkv, np.float32).astype(bf16))
    wproj_bf = np.ascontiguousarray(np.asarray(w_proj, np.float32).astype(bf16))
    bias_bf = np.ascontiguousarray(
        np.asarray(b_proj, np.float32).astype(bf16).reshape(1, D)
    )
    in_maps = []
    for i in range(NCORES):
        b, qh = divmod(i, 2)
        xtb = np.ascontiguousarray(x[b].T.astype(bf16))
        in_maps.append(
            {
                "xt": xtb,
                "xtq": np.ascontiguousarray(xtb[:, qh * QR : (qh + 1) * QR]),
                "wqkv": wqkv_bf,
                "wproj": wproj_bf,
                "biasb": bias_bf,
            }
        )

    res = run_bass_kernel_spmd(nc, in_maps, core_ids=list(range(NCORES)), trace=TRACE)
    LAST_EXEC_NS = res.exec_time_ns

    out = np.empty((B, N, D), np.float32)
    q_attn = np.empty((B, H, N), np.float32)
    for i, r in enumerate(res.results):
        b, qh = divmod(i, 2)
        out[b, qh * QR : (qh + 1) * QR] = np.asarray(r["y"], np.float32)
        if qh == 0:
            q_attn[b] = np.asarray(r["qattn"], np.float32)
    return out, q_attn


# revision 41
# speedup vs baseline: 1.0479x; 1.0479x over previous
"""Multi-head attention block (B=4, N=2048, D=768, H=12) on 8 TRN2 NeuronCores.

Sharding: core i -> batch b = i//2, query-row half qh = i%2 (1024 rows).
Each core computes all 12 heads for its (batch, q-half): qkv projection,
flash-style attention in S^T layout (keys on partitions, queries on free
axis), and the full output projection for its rows. No collectives: every
core produces complete output rows; host just concatenates.

Matmul inputs are bf16 (1 cyc/row on PE vs 4 for fp32); all accumulation is
fp32 in PSUM. Softmax skips the max-subtraction (scores are ~N(0,1); exp is
exact in fp32 for this range). Row-sums come from a ones-column appended to
V so the O-matmul accumulates them for free.

The emission order interleaves qkv column-tile work into the ACT-bound
attention kt-loops so the PE stream stays dense (keeps the HAM clock at
2.4 GHz) while the Scalar engine streams the exps.
"""

import sys

sys.path.insert(0, "/opt/trn_rl_repo")

import numpy as np
import ml_dtypes

import concourse.bass as bass
import concourse.tile as tile
from concourse import bacc, mybir
from concourse.bass_utils import run_bass_kernel_spmd
from concourse.masks import make_identity

B, N, D, H = 4, 2048, 768, 12
HD = D // H  # 64
SCALE = HD**-0.5
NCORES = 8
QR = N // 2  # q rows per core
KT = D // 128  # 6 contraction tiles
NT = N // 128  # 16 key-row tiles
BF = mybir.dt.bfloat16
F32 = mybir.dt.float32

TRACE = False
LAST_EXEC_NS = None
_CACHED_NC = None


def _body(tc, xt, xtq, wqkv, wproj, biasb, y, qattn):
    nc = tc.nc
    with (
        tc.tile_pool(name="const", bufs=1) as cp,
        tc.tile_pool(name="persist", bufs=1) as pp,
        # all PSUM lives in one scope: mm 2 banks + st 4 banks + o 2 banks
        tc.tile_pool(name="mmps", bufs=2, space="PSUM") as mmp,
        tc.tile_pool(name="stps", bufs=2, space="PSUM") as stp,
        tc.tile_pool(name="ops", bufs=1, space="PSUM") as op_,
        tc.tile_pool(name="work", bufs=3) as sb2,
    ):
        ones_bf = cp.tile([1, 128], BF)
        nc.vector.memset(ones_bf[:], 1.0)
        ident = cp.tile([128, 128], F32)
        make_identity(nc, ident[:])

        def emit_bias_bc():
            for n0, nw in [(0, 512), (512, 256)]:
                ps = mmp.tile([128, 512], F32, tag="mm", name="biasps")
                nc.tensor.matmul(
                    ps[:, :nw],
                    lhsT=ones_bf[:1, :128],
                    rhs=bias_sb[:1, n0 : n0 + nw],
                    start=True,
                    stop=True,
                )
                nc.vector.tensor_copy(bias_bc[:, n0 : n0 + nw], ps[:, :nw])
        bias_sb = cp.tile([1, D], BF)
        nc.sync.dma_start(bias_sb[:], biasb[:])
        bias_bc = cp.tile([128, D], F32)

        xt_sb = [pp.tile([128, N], BF, tag=f"xt{k}", name=f"xt{k}") for k in range(KT)]
        xtq_sb = [pp.tile([128, QR], BF, tag=f"xtq{k}", name=f"xtq{k}") for k in range(KT)]
        w_sb = [pp.tile([128, 3 * D], BF, tag=f"w{k}", name=f"w{k}") for k in range(KT)]
        wp_sb = [pp.tile([128, D], BF, tag=f"wp{k}", name=f"wp{k}") for k in range(KT)]
        qT_sb = [pp.tile([128, QR], BF, tag=f"qT{k}", name=f"qT{k}") for k in range(KT)]
        kT_sb = [pp.tile([128, N], BF, tag=f"kT{k}", name=f"kT{k}") for k in range(KT)]
        v_sb = [pp.tile([128, H * (HD + 1)], BF, tag=f"v{r}", name=f"v{r}") for r in range(NT)]
        oT_sb = [pp.tile([128, QR], BF, tag=f"oT{k}", name=f"oT{k}") for k in range(KT)]
        qattn_sb = pp.tile([16, H * 128], F32, tag="qattn_sb")

        # all input loads on the sync HWDGE queue, in need-first order:
        # xtq + w q/k-cols (gate qT/kT), then xt (kT/v), then v-cols/bias/wproj
        for k in range(KT):
            ks = slice(k * 128, (k + 1) * 128)
            nc.sync.dma_start(xtq_sb[k][:], xtq[ks, :])
            nc.sync.dma_start(w_sb[k][:, : 2 * D], wqkv[ks, : 2 * D])
        for k in range(KT):
            ks = slice(k * 128, (k + 1) * 128)
            nc.sync.dma_start(xt_sb[k][:], xt[ks, :])
        for k in range(KT):
            ks = slice(k * 128, (k + 1) * 128)
            nc.sync.dma_start(w_sb[k][:, 2 * D :], wqkv[ks, 2 * D :])
        for k in range(KT):
            ks = slice(k * 128, (k + 1) * 128)
            nc.sync.dma_start(wp_sb[k][:], wproj[ks, :])

        # ---- emit helpers ----

        def _drain(filler):
            while filler:
                try:
                    next(filler)()
                except StopIteration:
                    break

        def v_groups(r):
            # v in [row, feature] layout, ones column appended per head
            hv = v_sb[r][:].rearrange("p (h c) -> p h c", c=HD + 1)
            for gi, (c0, cw) in enumerate([(2 * D, 512), (2 * D + 512, 256)]):
                def emit(gi=gi, c0=c0, cw=cw):
                    if gi == 0:
                        nc.gpsimd.memset(hv[:, :, HD : HD + 1], 1.0)
                    ps = mmp.tile([128, 512], F32, tag="mm", name="vps")
                    for k in range(KT):
                        nc.tensor.matmul(
                            ps[:, :cw],
                            lhsT=xt_sb[k][:, r * 128 : (r + 1) * 128],
                            rhs=w_sb[k][:, c0 : c0 + cw],
                            start=(k == 0),
                            stop=(k == KT - 1),
                        )
                    h0 = (c0 - 2 * D) // HD
                    nc.vector.tensor_copy(
                        hv[:, h0 : h0 + cw // HD, 0:HD],
                        ps[:, :cw].rearrange("p (h c) -> p h c", c=HD),
                    )
                yield emit

        def qk_groups(c):
            # generator of closures: one psum-group (6 matmuls + copy) each
            is_q = c < 6
            rhs_tiles = xtq_sb if is_q else xt_sb
            nrows = QR if is_q else N
            dst = qT_sb[c] if is_q else kT_sb[c - 6]
            for n0 in range(0, nrows, 512):
                def emit(n0=n0):
                    ps = mmp.tile([128, 512], F32, tag="mm", name="qkps")
                    for k in range(KT):
                        nc.tensor.matmul(
                            ps[:],
                            lhsT=w_sb[k][:, c * 128 : (c + 1) * 128],
                            rhs=rhs_tiles[k][:, n0 : n0 + 512],
                            start=(k == 0),
                            stop=(k == KT - 1),
                        )
                    nc.vector.tensor_copy(dst[:, n0 : n0 + 512], ps[:])
                yield emit

        def emit_head(h, filler, every=3, per=1):
            # flash attention for one head; calls filler closures between kt
            # steps to keep the PE stream dense while ACT runs the exps.
            # Returns tail closures (PE-touching normalize/q_attn work) that
            # the caller should feed into the NEXT head's filler stream so
            # the PE queue never waits on the DVE reciprocal chain.
            t6 = h // 2
            po = (h % 2) * 64
            qT_h = qT_sb[t6][po : po + 64, :]  # [64, QR]
            o_ps = [
                op_.tile([HD + 1, 512], F32, tag=f"o{qc}", name=f"o{qc}")
                for qc in range(2)
            ]
            stage = sb2.tile([128, NT], F32, tag="stage")
            for kt in range(NT):
                st = stp.tile([128, QR], F32, tag="st")
                lhsT = kT_sb[t6][po : po + 64, kt * 128 : (kt + 1) * 128]
                for qc in range(2):
                    nc.tensor.matmul(
                        st[:, qc * 512 : (qc + 1) * 512],
                        lhsT=lhsT,
                        rhs=qT_h[:, qc * 512 : (qc + 1) * 512],
                        start=True,
                        stop=True,
                    )
                pT = sb2.tile([128, QR], BF, tag="pT", bufs=4)
                nc.scalar.activation(
                    pT[:], st[:], mybir.ActivationFunctionType.Exp, scale=SCALE
                )
                # filler fires BEFORE the O-matmuls: a filler writing v_sb[j]
                # emitted at kt=j still precedes O(j) in program order
                if kt % every == 0 and filler:
                    for _ in range(per):
                        try:
                            next(filler)()
                        except StopIteration:
                            filler = None
                            break
                vh = v_sb[kt][:, h * (HD + 1) : (h + 1) * (HD + 1)]  # [128, 65]
                for qc in range(2):
                    nc.tensor.matmul(
                        o_ps[qc][:],
                        lhsT=vh,
                        rhs=pT[:, qc * 512 : (qc + 1) * 512],
                        start=(kt == 0),
                        stop=(kt == NT - 1),
                    )
                nc.gpsimd.tensor_copy(stage[:, kt : kt + 1], pT[:, 0:1])
            # evacuate o_ps fast (frees accumulator banks for the next head)
            # and run the DVE-only reciprocal chain now; defer PE-touching
            # normalize/q_attn ops as closures.
            o_raw = sb2.tile([HD + 1, QR], F32, tag="o_raw", bufs=2)
            for qc in range(2):
                nc.vector.tensor_copy(o_raw[:, qc * 512 : (qc + 1) * 512], o_ps[qc][:])
            rowsum = sb2.tile([1, QR], F32, tag="rowsum")
            nc.vector.tensor_copy(rowsum[:], o_raw[HD : HD + 1, :])
            recip = sb2.tile([1, QR], F32, tag="recip")
            nc.vector.reciprocal_approx_fast(recip[:], rowsum[:])
            recip_bf = sb2.tile([1, QR], BF, tag="recip_bf")
            nc.vector.tensor_copy(recip_bf[:], recip[:])

            scale_sb = sb2.tile([128, 1], F32, tag="scale_sb")

            def norm_qc(qc):
                def emit():
                    bc = mmp.tile([128, 512], F32, tag="mm", name="bc")
                    nc.tensor.matmul(
                        bc[:],
                        lhsT=ones_bf[:1, :128],
                        rhs=recip_bf[:1, qc * 512 : (qc + 1) * 512],
                        start=True,
                        stop=True,
                    )
                    nc.vector.tensor_mul(
                        oT_sb[t6][po : po + 64, qc * 512 : (qc + 1) * 512],
                        o_raw[0:HD, qc * 512 : (qc + 1) * 512],
                        bc[0:HD, :],
                    )
                    if qc == 0:
                        # column 0 = 1/rowsum(q=0) on all 128 partitions:
                        # doubles as the q_attn scale factor
                        nc.vector.tensor_copy(scale_sb[:], bc[:, 0:1])
                return emit

            def qattn_scale():
                # q_attn: query row 0's softmax row (valid on qh==0 cores)
                nc.vector.tensor_scalar_mul(stage[:], stage[:], scale_sb[:])

            def qattn_store():
                tp = mmp.tile([128, 512], F32, tag="mm", name="tp")
                nc.tensor.transpose(tp[0:NT, 0:128], stage[:], ident[:])
                nc.vector.tensor_copy(
                    qattn_sb[:, h * 128 : (h + 1) * 128], tp[0:NT, 0:128]
                )

            return filler, [norm_qc(0), norm_qc(1), qattn_scale, qattn_store]

        # ---- emission: first q/k columns + 3 v tiles, then heads; filler
        # streams the remaining v tiles, prior head's tail, next q/k cols ----
        qk0 = list(qk_groups(0))
        qk6 = list(qk_groups(6))
        for g in [*qk0, qk6[0]]:
            g()
        pending = []
        for p in range(6):
            for h in (2 * p, 2 * p + 1):
                if h == 0:
                    fl = iter(
                        [
                            *v_groups(0),
                            *v_groups(1),
                            *qk6[1:],
                            *(g for r in range(2, NT) for g in v_groups(r)),
                            emit_bias_bc,
                        ]
                    )
                    fl, tail = emit_head(0, fl, every=1, per=3)
                    _drain(fl)
                    pending = [*tail, *qk_groups(1), *qk_groups(7)]
                else:
                    if h % 2 == 0 and p < 5:
                        pending += [*qk_groups(p + 1), *qk_groups(7 + p)]
                    fl = iter(pending)
                    fl, tail = emit_head(h, fl, every=2, per=1)
                    leftover = list(fl) if fl else []
                    pending = leftover + list(tail)
        for g in pending:
            g()

        nc.sync.dma_start(
            qattn.rearrange("h (t c) -> t h c", c=128),
            qattn_sb[:].rearrange("p (h c) -> p h c", c=128),
        )

        # ---- output projection (+bias via ones row) ----
        for r in range(QR // 128):
            ytile = sb2.tile([128, D], BF, tag="y", bufs=2)
            for gi, (n0, nw) in enumerate([(0, 512), (512, 256)]):
                pool, tg = [(mmp, "mm"), (op_, "o0"), (op_, "o1")][(2 * r + gi) % 3]
                ps = pool.tile([128, 512], F32, tag=tg, name="pjps")
                for k in range(KT):
                    nc.tensor.matmul(
                        ps[:, :nw],
                        lhsT=oT_sb[k][:, r * 128 : (r + 1) * 128],
                        rhs=wp_sb[k][:, n0 : n0 + nw],
                        start=(k == 0),
                        stop=(k == KT - 1),
                    )
                nc.vector.tensor_add(
                    ytile[:, n0 : n0 + nw], ps[:, :nw], bias_bc[:, n0 : n0 + nw]
                )
            nc.sync.dma_start(y[r * 128 : (r + 1) * 128, :], ytile[:])


def _build():
    nc = bacc.Bacc("TRN2", target_bir_lowering=False, debug=False, num_devices=NCORES)
    xt = nc.declare_dram_parameter("xt", [D, N], BF, isOutput=False)
    xtq = nc.declare_dram_parameter("xtq", [D, QR], BF, isOutput=False)
    wqkv = nc.declare_dram_parameter("wqkv", [D, 3 * D], BF, isOutput=False)
    wproj = nc.declare_dram_parameter("wproj", [D, D], BF, isOutput=False)
    biasb = nc.declare_dram_parameter("biasb", [1, D], BF, isOutput=False)
    y = nc.declare_dram_parameter("y", [QR, D], BF, isOutput=True)
    qattn = nc.declare_dram_parameter("qattn", [H, N], F32, isOutput=True)
    with tile.TileContext(nc) as tc:
        _body(tc, xt.ap(), xtq.ap(), wqkv.ap(), wproj.ap(), biasb.ap(), y.ap(), qattn.ap())
    nc.finalize()
    return nc


def kernel(x, w_qkv, w_proj, b_proj):
    global _CACHED_NC, LAST_EXEC_NS
    if _CACHED_NC is None:
        _CACHED_NC = _build()
    nc = _CACHED_NC

    bf16 = ml_dtypes.bfloat16
    x = np.asarray(x, np.float32)
    wqkv_bf = np.ascontiguousarray(np.asarray(w_qkv, np.float32).astype(bf16))
    wproj_bf = np.ascontiguousarray(np.asarray(w_proj, np.float32).astype(bf16))
    bias_bf = np.ascontiguousarray(
        np.asarray(b_proj, np.float32).astype(bf16).reshape(1, D)
    )
    in_maps = []
    for i in range(NCORES):
        b, qh = divmod(i, 2)
        xtb = np.ascontiguousarray(x[b].T.astype(bf16))
        in_maps.append(
            {
                "xt": xtb,
                "xtq": np.ascontiguousarray(xtb[:, qh * QR : (qh + 1) * QR]),
                "wqkv": wqkv_bf,
                "wproj": wproj_bf,
                "biasb": bias_bf,
            }
        )

    res = run_bass_kernel_spmd(nc, in_maps, core_ids=list(range(NCORES)), trace=TRACE)
    LAST_EXEC_NS = res.exec_time_ns

    out = np.empty((B, N, D), np.float32)
    q_attn = np.empty((B, H, N), np.float32)
    for i, r in enumerate(res.results):
        b, qh = divmod(i, 2)
        out[b, qh * QR : (qh + 1) * QR] = np.asarray(r["y"], np.float32)
        if qh == 0:
            q_attn[b] = np.asarray(r["qattn"], np.float32)
    return out, q_attn


# revision 42
# speedup vs baseline: 1.0526x; 1.0045x over previous
"""Multi-head attention block (B=4, N=2048, D=768, H=12) on 8 TRN2 NeuronCores.

Sharding: core i -> batch b = i//2, query-row half qh = i%2 (1024 rows).
Each core computes all 12 heads for its (batch, q-half): qkv projection,
flash-style attention in S^T layout (keys on partitions, queries on free
axis), and the full output projection for its rows. No collectives: every
core produces complete output rows; host just concatenates.

Matmul inputs are bf16 (1 cyc/row on PE vs 4 for fp32); all accumulation is
fp32 in PSUM. Softmax skips the max-subtraction (scores are ~N(0,1); exp is
exact in fp32 for this range). Row-sums come from a ones-column appended to
V so the O-matmul accumulates them for free.

The emission order interleaves qkv column-tile work into the ACT-bound
attention kt-loops so the PE stream stays dense (keeps the HAM clock at
2.4 GHz) while the Scalar engine streams the exps.
"""

import sys

sys.path.insert(0, "/opt/trn_rl_repo")

import numpy as np
import ml_dtypes

import concourse.bass as bass
import concourse.tile as tile
from concourse import bacc, mybir
from concourse.bass_utils import run_bass_kernel_spmd
from concourse.masks import make_identity

B, N, D, H = 4, 2048, 768, 12
HD = D // H  # 64
SCALE = HD**-0.5
NCORES = 8
QR = N // 2  # q rows per core
KT = D // 128  # 6 contraction tiles
NT = N // 128  # 16 key-row tiles
BF = mybir.dt.bfloat16
F32 = mybir.dt.float32

TRACE = False
LAST_EXEC_NS = None
_CACHED_NC = None


def _body(tc, xt, xtq, wqkv, wproj, biasb, y, qattn):
    nc = tc.nc
    with (
        tc.tile_pool(name="const", bufs=1) as cp,
        tc.tile_pool(name="persist", bufs=1) as pp,
        # all PSUM lives in one scope: mm 2 banks + st 4 banks + o 2 banks
        tc.tile_pool(name="mmps", bufs=2, space="PSUM") as mmp,
        tc.tile_pool(name="stps", bufs=2, space="PSUM") as stp,
        tc.tile_pool(name="ops", bufs=1, space="PSUM") as op_,
        tc.tile_pool(name="work", bufs=3) as sb2,
    ):
        ones_bf = cp.tile([1, 128], BF)
        nc.vector.memset(ones_bf[:], 1.0)
        ident = cp.tile([128, 128], F32)
        make_identity(nc, ident[:])

        def emit_bias_bc():
            for n0, nw in [(0, 512), (512, 256)]:
                ps = mmp.tile([128, 512], F32, tag="mm", name="biasps")
                nc.tensor.matmul(
                    ps[:, :nw],
                    lhsT=ones_bf[:1, :128],
                    rhs=bias_sb[:1, n0 : n0 + nw],
                    start=True,
                    stop=True,
                )
                nc.vector.tensor_copy(bias_bc[:, n0 : n0 + nw], ps[:, :nw])
        bias_sb = cp.tile([1, D], BF)
        nc.sync.dma_start(bias_sb[:], biasb[:])
        bias_bc = cp.tile([128, D], F32)

        xt_sb = [pp.tile([128, N], BF, tag=f"xt{k}", name=f"xt{k}") for k in range(KT)]
        xtq_sb = [pp.tile([128, QR], BF, tag=f"xtq{k}", name=f"xtq{k}") for k in range(KT)]
        w_sb = [pp.tile([128, 3 * D], BF, tag=f"w{k}", name=f"w{k}") for k in range(KT)]
        wp_sb = [pp.tile([128, D], BF, tag=f"wp{k}", name=f"wp{k}") for k in range(KT)]
        qT_sb = [pp.tile([128, QR], BF, tag=f"qT{k}", name=f"qT{k}") for k in range(KT)]
        kT_sb = [pp.tile([128, N], BF, tag=f"kT{k}", name=f"kT{k}") for k in range(KT)]
        v_sb = [pp.tile([128, H * (HD + 1)], BF, tag=f"v{r}", name=f"v{r}") for r in range(NT)]
        oT_sb = [pp.tile([128, QR], BF, tag=f"oT{k}", name=f"oT{k}") for k in range(KT)]
        qattn_sb = pp.tile([16, H * 128], F32, tag="qattn_sb")

        # all input loads on the sync HWDGE queue, in need-first order:
        # xtq + w q/k-cols (gate qT/kT), then xt (kT/v), then v-cols/bias/wproj
        for k in range(KT):
            ks = slice(k * 128, (k + 1) * 128)
            nc.sync.dma_start(xtq_sb[k][:], xtq[ks, :])
            nc.sync.dma_start(w_sb[k][:, : 2 * D], wqkv[ks, : 2 * D])
        for k in range(KT):
            ks = slice(k * 128, (k + 1) * 128)
            nc.sync.dma_start(xt_sb[k][:], xt[ks, :])
        for k in range(KT):
            ks = slice(k * 128, (k + 1) * 128)
            nc.sync.dma_start(w_sb[k][:, 2 * D :], wqkv[ks, 2 * D :])
        for k in range(KT):
            ks = slice(k * 128, (k + 1) * 128)
            nc.sync.dma_start(wp_sb[k][:], wproj[ks, :])

        # ---- emit helpers ----

        def _drain(filler):
            while filler:
                try:
                    next(filler)()
                except StopIteration:
                    break

        def v_groups(r):
            # v in [row, feature] layout, ones column appended per head
            hv = v_sb[r][:].rearrange("p (h c) -> p h c", c=HD + 1)
            for gi, (c0, cw) in enumerate([(2 * D, 512), (2 * D + 512, 256)]):
                def emit(gi=gi, c0=c0, cw=cw):
                    if gi == 0:
                        nc.gpsimd.memset(hv[:, :, HD : HD + 1], 1.0)
                    ps = mmp.tile([128, 512], F32, tag="mm", name="vps")
                    for k in range(KT):
                        nc.tensor.matmul(
                            ps[:, :cw],
                            lhsT=xt_sb[k][:, r * 128 : (r + 1) * 128],
                            rhs=w_sb[k][:, c0 : c0 + cw],
                            start=(k == 0),
                            stop=(k == KT - 1),
                        )
                    h0 = (c0 - 2 * D) // HD
                    nh = cw // HD
                    nc.vector.tensor_copy(
                        hv[:, h0 : h0 + nh // 2, 0:HD],
                        ps[:, : cw // 2].rearrange("p (h c) -> p h c", c=HD),
                    )
                    nc.vector.tensor_copy(
                        hv[:, h0 + nh // 2 : h0 + nh, 0:HD],
                        ps[:, cw // 2 : cw].rearrange("p (h c) -> p h c", c=HD),
                    )
                yield emit

        def qk_groups(c):
            # generator of closures: one psum-group (6 matmuls + copy) each
            is_q = c < 6
            rhs_tiles = xtq_sb if is_q else xt_sb
            nrows = QR if is_q else N
            dst = qT_sb[c] if is_q else kT_sb[c - 6]
            for n0 in range(0, nrows, 512):
                def emit(n0=n0):
                    ps = mmp.tile([128, 512], F32, tag="mm", name="qkps")
                    for k in range(KT):
                        nc.tensor.matmul(
                            ps[:],
                            lhsT=w_sb[k][:, c * 128 : (c + 1) * 128],
                            rhs=rhs_tiles[k][:, n0 : n0 + 512],
                            start=(k == 0),
                            stop=(k == KT - 1),
                        )
                    nc.vector.tensor_copy(dst[:, n0 : n0 + 256], ps[:, 0:256])
                    nc.vector.tensor_copy(dst[:, n0 + 256 : n0 + 512], ps[:, 256:512])
                yield emit

        def emit_head(h, filler, every=3, per=1):
            # flash attention for one head; calls filler closures between kt
            # steps to keep the PE stream dense while ACT runs the exps.
            # Returns tail closures (PE-touching normalize/q_attn work) that
            # the caller should feed into the NEXT head's filler stream so
            # the PE queue never waits on the DVE reciprocal chain.
            t6 = h // 2
            po = (h % 2) * 64
            qT_h = qT_sb[t6][po : po + 64, :]  # [64, QR]
            o_ps = [
                op_.tile([HD + 1, 512], F32, tag=f"o{qc}", name=f"o{qc}")
                for qc in range(2)
            ]
            stage = sb2.tile([128, NT], F32, tag="stage")
            for kt in range(NT):
                st = stp.tile([128, QR], F32, tag="st")
                lhsT = kT_sb[t6][po : po + 64, kt * 128 : (kt + 1) * 128]
                for qc in range(2):
                    nc.tensor.matmul(
                        st[:, qc * 512 : (qc + 1) * 512],
                        lhsT=lhsT,
                        rhs=qT_h[:, qc * 512 : (qc + 1) * 512],
                        start=True,
                        stop=True,
                    )
                pT = sb2.tile([128, QR], BF, tag="pT", bufs=4)
                nc.scalar.activation(
                    pT[:], st[:], mybir.ActivationFunctionType.Exp, scale=SCALE
                )
                # filler fires BEFORE the O-matmuls: a filler writing v_sb[j]
                # emitted at kt=j still precedes O(j) in program order
                if kt % every == 0 and filler:
                    for _ in range(per):
                        try:
                            next(filler)()
                        except StopIteration:
                            filler = None
                            break
                vh = v_sb[kt][:, h * (HD + 1) : (h + 1) * (HD + 1)]  # [128, 65]
                for qc in range(2):
                    nc.tensor.matmul(
                        o_ps[qc][:],
                        lhsT=vh,
                        rhs=pT[:, qc * 512 : (qc + 1) * 512],
                        start=(kt == 0),
                        stop=(kt == NT - 1),
                    )
                nc.gpsimd.tensor_copy(stage[:, kt : kt + 1], pT[:, 0:1])
            # evacuate o_ps fast (frees accumulator banks for the next head)
            # and run the DVE-only reciprocal chain now; defer PE-touching
            # normalize/q_attn ops as closures.
            o_raw = sb2.tile([HD + 1, QR], F32, tag="o_raw", bufs=2)
            for qc in range(2):
                nc.vector.tensor_copy(o_raw[:, qc * 512 : (qc + 1) * 512], o_ps[qc][:])
            rowsum = sb2.tile([1, QR], F32, tag="rowsum")
            nc.vector.tensor_copy(rowsum[:], o_raw[HD : HD + 1, :])
            recip = sb2.tile([1, QR], F32, tag="recip")
            nc.vector.reciprocal_approx_fast(recip[:], rowsum[:])
            recip_bf = sb2.tile([1, QR], BF, tag="recip_bf")
            nc.vector.tensor_copy(recip_bf[:], recip[:])

            scale_sb = sb2.tile([128, 1], F32, tag="scale_sb")

            def norm_qc(qc):
                def emit():
                    bc = mmp.tile([128, 512], F32, tag="mm", name="bc")
                    nc.tensor.matmul(
                        bc[:],
                        lhsT=ones_bf[:1, :128],
                        rhs=recip_bf[:1, qc * 512 : (qc + 1) * 512],
                        start=True,
                        stop=True,
                    )
                    nc.vector.tensor_mul(
                        oT_sb[t6][po : po + 64, qc * 512 : (qc + 1) * 512],
                        o_raw[0:HD, qc * 512 : (qc + 1) * 512],
                        bc[0:HD, :],
                    )
                    if qc == 0:
                        # column 0 = 1/rowsum(q=0) on all 128 partitions:
                        # doubles as the q_attn scale factor
                        nc.vector.tensor_copy(scale_sb[:], bc[:, 0:1])
                return emit

            def qattn_scale():
                # q_attn: query row 0's softmax row (valid on qh==0 cores)
                nc.vector.tensor_scalar_mul(stage[:], stage[:], scale_sb[:])

            def qattn_store():
                tp = mmp.tile([128, 512], F32, tag="mm", name="tp")
                nc.tensor.transpose(tp[0:NT, 0:128], stage[:], ident[:])
                nc.vector.tensor_copy(
                    qattn_sb[:, h * 128 : (h + 1) * 128], tp[0:NT, 0:128]
                )

            return filler, [norm_qc(0), norm_qc(1), qattn_scale, qattn_store]

        # ---- emission: first q/k columns + 3 v tiles, then heads; filler
        # streams the remaining v tiles, prior head's tail, next q/k cols ----
        qk0 = list(qk_groups(0))
        qk6 = list(qk_groups(6))
        for g in [*qk0, qk6[0]]:
            g()
        pending = []
        for p in range(6):
            for h in (2 * p, 2 * p + 1):
                if h == 0:
                    fl = iter(
                        [
                            *v_groups(0),
                            *v_groups(1),
                            *qk6[1:],
                            *(g for r in range(2, NT) for g in v_groups(r)),
                            emit_bias_bc,
                        ]
                    )
                    fl, tail = emit_head(0, fl, every=1, per=3)
                    _drain(fl)
                    pending = [*tail, *qk_groups(1), *qk_groups(7)]
                else:
                    if h % 2 == 0 and p < 5:
                        pending += [*qk_groups(p + 1), *qk_groups(7 + p)]
                    fl = iter(pending)
                    fl, tail = emit_head(h, fl, every=2, per=1)
                    leftover = list(fl) if fl else []
                    pending = leftover + list(tail)
        for g in pending:
            g()

        nc.sync.dma_start(
            qattn.rearrange("h (t c) -> t h c", c=128),
            qattn_sb[:].rearrange("p (h c) -> p h c", c=128),
        )

        # ---- output projection (+bias via ones row) ----
        for r in range(QR // 128):
            ytile = sb2.tile([128, D], BF, tag="y", bufs=2)
            for gi, (n0, nw) in enumerate([(0, 512), (512, 256)]):
                pool, tg = [(mmp, "mm"), (op_, "o0"), (op_, "o1")][(2 * r + gi) % 3]
                ps = pool.tile([128, 512], F32, tag=tg, name="pjps")
                for k in range(KT):
                    nc.tensor.matmul(
                        ps[:, :nw],
                        lhsT=oT_sb[k][:, r * 128 : (r + 1) * 128],
                        rhs=wp_sb[k][:, n0 : n0 + nw],
                        start=(k == 0),
                        stop=(k == KT - 1),
                    )
                nc.vector.tensor_add(
                    ytile[:, n0 : n0 + nw], ps[:, :nw], bias_bc[:, n0 : n0 + nw]
                )
            nc.sync.dma_start(y[r * 128 : (r + 1) * 128, :], ytile[:])


def _build():
    nc = bacc.Bacc("TRN2", target_bir_lowering=False, debug=False, num_devices=NCORES)
    xt = nc.declare_dram_parameter("xt", [D, N], BF, isOutput=False)
    xtq = nc.declare_dram_parameter("xtq", [D, QR], BF, isOutput=False)
    wqkv = nc.declare_dram_parameter("wqkv", [D, 3 * D], BF, isOutput=False)
    wproj = nc.declare_dram_parameter("wproj", [D, D], BF, isOutput=False)
    biasb = nc.declare_dram_parameter("biasb", [1, D], BF, isOutput=False)
    y = nc.declare_dram_parameter("y", [QR, D], BF, isOutput=True)
    qattn = nc.declare_dram_parameter("qattn", [H, N], F32, isOutput=True)
    with tile.TileContext(nc) as tc:
        _body(tc, xt.ap(), xtq.ap(), wqkv.ap(), wproj.ap(), biasb.ap(), y.ap(), qattn.ap())
    nc.finalize()
    return nc


def kernel(x, w_qkv, w_proj, b_proj):
    global _CACHED_NC, LAST_EXEC_NS
    if _CACHED_NC is None:
        _CACHED_NC = _build()
    nc = _CACHED_NC

    bf16 = ml_dtypes.bfloat16
    x = np.asarray(x, np.float32)
    wqkv_bf = np.ascontiguousarray(np.asarray(w_qkv, np.float32).astype(bf16))
    wproj_bf = np.ascontiguousarray(np.asarray(w_proj, np.float32).astype(bf16))
    bias_bf = np.ascontiguousarray(
        np.asarray(b_proj, np.float32).astype(bf16).reshape(1, D)
    )
    in_maps = []
    for i in range(NCORES):
        b, qh = divmod(i, 2)
        xtb = np.ascontiguousarray(x[b].T.astype(bf16))
        in_maps.append(
            {
                "xt": xtb,
                "xtq": np.ascontiguousarray(xtb[:, qh * QR : (qh + 1) * QR]),
                "wqkv": wqkv_bf,
                "wproj": wproj_bf,
                "biasb": bias_bf,
            }
        )

    res = run_bass_kernel_spmd(nc, in_maps, core_ids=list(range(NCORES)), trace=TRACE)
    LAST_EXEC_NS = res.exec_time_ns

    out = np.empty((B, N, D), np.float32)
    q_attn = np.empty((B, H, N), np.float32)
    for i, r in enumerate(res.results):
        b, qh = divmod(i, 2)
        out[b, qh * QR : (qh + 1) * QR] = np.asarray(r["y"], np.float32)
        if qh == 0:
            q_attn[b] = np.asarray(r["qattn"], np.float32)
    return out, q_attn
